# revision 1
# baseline (speedup 1.0000x reference)
"""CrossCondGPT2 forward on 8 trn2 NeuronCores.

Sharding: 4-way data parallel over batch B=4 x 2-way split of T=1024 within
each batch element (cores 2b, 2b+1 own rows [0,512) and [512,1024) of batch b).

Per layer, per core (R=512 own rows, fp32 residual h in SBUF):
  LN1 (row-major, bn_stats) -> a_own [R,C]
  PE-transpose -> aT_own [C,R] (feature-major, fp32r)
  pair ReduceScatter(aT_own duplicated) -> aT_sum; aT_peer = aT_sum - aT_own
    (local key order [own rows, peer rows]; the cross-cond mask pattern
     (i%256 >= j%256) is invariant to the 512-row swap since 512 % 256 == 0)
  qT = Wq_eff-chunks @ aT_own (feature-major; 1/8 scale + ln1_g folded into W)
  kT = Wk_eff-chunks @ [aT_own | aT_peer] (full 1024 keys)
  V row-major (aT chunks as lhsT), evicted strided into V_aug [keys,12,65]
    with a ones column so AV also produces softmax sums
  per head: scoresT [keys, q] blocks, skipping all-masked blocks (25%);
    exp without max-subtraction (|scores| <= ~4 by construction);
    triu mask-mult on diagonal blocks; AV: yT[65,512] = V_aug^T @ expT with
    sums in row 64; divide via PE-broadcast of reciprocal sums
  proj row-major (lhsT = yT chunks), h += proj
  LN2 -> mT; fc1 (gelu+bias on ACT evict, ln2_g folded); fc2 of-outer in two
    column passes; h += mlp

kernel(**inputs) takes FULL unsharded inputs, returns [B,T,C] fp32.
"""

import sys

if "/opt/trn_rl_repo" not in sys.path:
    sys.path.insert(0, "/opt/trn_rl_repo")

import numpy as np

import concourse.bacc as bacc
import concourse.mybir as mybir
import concourse.tile as tile

L, B, T, C, NH, HD, HID = 4, 4, 1024, 768, 12, 64, 3072
R = 512  # rows per core
P = 128
NCORES = 8
PAIRS = [[0, 1], [2, 3], [4, 5], [6, 7]]
F32, F32R = mybir.dt.float32, mybir.dt.float32r
AF = mybir.ActivationFunctionType
ALU = mybir.AluOpType
EPS = 1e-5

CT = C // P  # 6 feature chunks of 128
RT = R // P  # 4 own row tiles
KT = T // P  # 8 key chunks
HT = HID // P  # 24 hidden chunks


def round_f32r(x):
    """Round fp32 -> fp32r-representable (bf16-pair; adjust if HW differs)."""
    import ml_dtypes

    x = np.ascontiguousarray(x, dtype=np.float32)
    hi = x.astype(ml_dtypes.bfloat16).astype(np.float32)
    lo = (x - hi).astype(ml_dtypes.bfloat16).astype(np.float32)
    return hi + lo


def build(zq, zp, z2):
    """zq/zp/z2: skip qkv / proj / fc2 bias paths when those biases are zero."""
    nc = bacc.Bacc(None, target_bir_lowering=False, debug=False)

    x_in = nc.declare_dram_parameter("x", [R, C], F32, isOutput=False)
    wq_in = nc.declare_dram_parameter("wq", [L, C, C], F32R, isOutput=False)
    wk_in = nc.declare_dram_parameter("wk", [L, C, C], F32R, isOutput=False)
    wv_in = nc.declare_dram_parameter("wv", [L, C, C], F32R, isOutput=False)
    wp_in = nc.declare_dram_parameter("wp", [L, C, C], F32R, isOutput=False)
    w1_in = nc.declare_dram_parameter("w1", [L, C, HID], F32R, isOutput=False)
    w2_in = nc.declare_dram_parameter("w2", [L, HID, C], F32R, isOutput=False)
    bq_in = nc.declare_dram_parameter("bq", [L, C], F32, isOutput=False)
    bk_in = nc.declare_dram_parameter("bk", [L, C], F32, isOutput=False)
    bv_in = nc.declare_dram_parameter("bv", [L, C], F32R, isOutput=False)
    bp_in = nc.declare_dram_parameter("bp", [L, C], F32R, isOutput=False)
    b1_in = nc.declare_dram_parameter("b1", [L, HID], F32, isOutput=False)
    b2_in = nc.declare_dram_parameter("b2", [L, C], F32R, isOutput=False)
    triu_in = nc.declare_dram_parameter("triu", [P, P], F32R, isOutput=False)
    ident_in = nc.declare_dram_parameter("ident", [P, P], F32, isOutput=False)
    ones_in = nc.declare_dram_parameter("ones_row", [1, P], F32R, isOutput=False)
    out_d = nc.declare_dram_parameter("out", [R, C], F32, isOutput=True)

    with tile.TileContext(nc) as tc:
        with (
            tc.tile_pool(name="res", bufs=1) as res,
            tc.tile_pool(name="wqkv", bufs=7) as wqkv_p,
            tc.tile_pool(name="w1p", bufs=7) as w1_p,
            tc.tile_pool(name="w2p", bufs=3) as w2_p,
            tc.tile_pool(name="b2k", bufs=24) as b2k,
            tc.tile_pool(name="kt", bufs=6) as kt_p,
            tc.tile_pool(name="vaug", bufs=8) as vaug_p,
            tc.tile_pool(name="rot", bufs=3) as rot,
            tc.tile_pool(name="attn", bufs=7) as attn_p,
            tc.tile_pool(name="small", bufs=2) as small,
            tc.tile_pool(name="ps", bufs=3, space="PSUM") as ps,
            tc.tile_pool(name="psacc", bufs=4, space="PSUM") as psacc,
            tc.tile_pool(name="dram", bufs=2, space="DRAM") as dram,
        ):
            # ---- persistent tiles ----
            ident = res.tile([P, P], F32, tag="ident")
            nc.sync.dma_start(ident[:], ident_in[:])
            triu = res.tile([P, P], F32R, tag="triu")
            nc.sync.dma_start(triu[:], triu_in[:])
            ones_row = res.tile([1, P], F32R, tag="ones_row")
            nc.sync.dma_start(ones_row[:], ones_in[:])
            eps_t = res.tile([P, 1], F32, tag="eps")
            nc.vector.memset(eps_t[:], EPS)
            ones_pp = res.tile([P, NH], F32, tag="ones_pp")
            nc.vector.memset(ones_pp[:], 1.0)
            zero_t = res.tile([P, P], F32, tag="zero_t")
            nc.vector.memset(zero_t[:], 0.0)

            h = []
            for rt in range(RT):
                ht_ = res.tile([P, C], F32, tag=f"h{rt}")
                nc.sync.dma_start(ht_[:], x_in[rt * P : (rt + 1) * P, :])
                h.append(ht_)

            def layernorm(rt):
                """Row-major LN of h[rt] (gain/bias folded downstream)."""
                a_t = rot.tile([P, C], F32, tag="ln_out", bufs=2)
                stats = small.tile([P, 3, 6], F32, tag="bn_stats")
                xg = h[rt][:].rearrange("p (g d) -> p g d", g=3)
                for g in range(3):
                    nc.vector.bn_stats(stats[:, g, :], xg[:, g, :])
                mv = small.tile([P, 2], F32, tag="bn_mv")
                nc.vector.bn_aggr(mv[:], stats[:])
                std = small.tile([P, 1], F32, tag="bn_std")
                nc.scalar.activation(
                    std[:], mv[:, 1:2], AF.Sqrt, bias=eps_t[:], scale=1.0
                )
                rstd = small.tile([P, 1], F32, tag="bn_rstd")
                nc.vector.reciprocal(rstd[:], std[:])
                nc.vector.tensor_scalar(
                    out=a_t[:],
                    in0=h[rt][:],
                    scalar1=mv[:, 0:1],
                    scalar2=rstd[:],
                    op0=ALU.subtract,
                    op1=ALU.mult,
                )
                return a_t

            def transpose_to_feat(pool, tag, bufs):
                """LN all 4 row tiles -> 6 fp32r feature-major [P, R] tiles."""
                ft = [
                    pool.tile([P, R], F32R, tag=tag, bufs=bufs, name=f"ft{fc}")
                    for fc in range(CT)
                ]
                for rt in range(RT):
                    a_t = layernorm(rt)
                    for fc in range(CT):
                        tp = ps.tile([P, P], F32, tag="ps")
                        nc.tensor.transpose(
                            tp[:], a_t[:, fc * P : (fc + 1) * P], ident[:]
                        )
                        nc.scalar.activation(
                            ft[fc][:, rt * P : (rt + 1) * P], tp[:], AF.Copy
                        )
                return ft

            for layer in range(L):
                lsl = slice(layer, layer + 1)

                # ---- per-layer weight/bias loads (qkv) ----
                wq = [wqkv_p.tile([P, C], F32R, tag="wqkv", name=f"wq{i}") for i in range(CT)]
                for i in range(CT):
                    nc.sync.dma_start(wq[i][:], wq_in[layer, i * P : (i + 1) * P, :])
                wk = [wqkv_p.tile([P, C], F32R, tag="wqkv", name=f"wk{i}") for i in range(CT)]
                for i in range(CT):
                    nc.sync.dma_start(wk[i][:], wk_in[layer, i * P : (i + 1) * P, :])
                wv = [wqkv_p.tile([P, C], F32R, tag="wqkv", name=f"wv{i}") for i in range(CT)]
                for i in range(CT):
                    nc.sync.dma_start(wv[i][:], wv_in[layer, i * P : (i + 1) * P, :])

                if not zq:
                    bq_sb = small.tile([P, CT], F32, tag="bq_sb")
                    bk_sb = small.tile([P, CT], F32, tag="bk_sb")
                    bv_row = small.tile([1, C], F32R, tag="bv_row")
                    nc.sync.dma_start(
                        bq_sb[:], bq_in[lsl, :].rearrange("o (f p) -> p (o f)", p=P)
                    )
                    nc.sync.dma_start(
                        bk_sb[:], bk_in[lsl, :].rearrange("o (f p) -> p (o f)", p=P)
                    )
                    nc.sync.dma_start(bv_row[:], bv_in[lsl, :])

                # ---- LN1 + transpose own rows ----
                aT = transpose_to_feat(b2k, "b2k", 24)

                # ---- pair exchange: ReduceScatter(dup) -> sum; peer=sum-own
                rs_in = dram.tile([2, CT, P, R], F32R, tag="rs_in")
                rs_out = dram.tile([CT, P, R], F32R, tag="rs_out")
                for fc in range(CT):
                    nc.gpsimd.dma_start(rs_in[0, fc], aT[fc][:])
                    nc.gpsimd.dma_start(rs_in[1, fc], aT[fc][:])
                nc.gpsimd.collective_compute(
                    "ReduceScatter",
                    ALU.add,
                    replica_groups=PAIRS,
                    ins=[rs_in[:]],
                    outs=[rs_out[:]],
                )
                aTp = []
                for fc in range(CT):
                    s_t = rot.tile([P, R], F32, tag="aT_sum", bufs=2)
                    nc.gpsimd.dma_start(s_t[:], rs_out[fc].bitcast(F32))
                    p_t = b2k.tile([P, R], F32R, tag="b2k", bufs=24, name=f"aTp{fc}")
                    nc.vector.tensor_tensor(
                        out=p_t[:],
                        in0=s_t[:],
                        in1=aT[fc][:].bitcast(F32),
                        op=ALU.subtract,
                    )
                    aTp.append(p_t)

                # ---- qT (own rows only) ----
                qT = [b2k.tile([P, R], F32R, tag="b2k", bufs=24, name=f"qT{i}") for i in range(CT)]
                for of in range(CT):
                    pq = ps.tile([P, R], F32, tag="ps")
                    for i in range(CT):
                        nc.tensor.matmul(
                            pq[:],
                            wq[i][:, of * P : (of + 1) * P],
                            aT[i][:],
                            start=(i == 0),
                            stop=(i == CT - 1),
                        )
                    if zq:
                        nc.scalar.activation(qT[of][:], pq[:], AF.Copy)
                    else:
                        nc.scalar.activation(
                            qT[of][:], pq[:], AF.Identity, bias=bq_sb[:, of : of + 1]
                        )

                # ---- kT (full keys, local order [own, peer]) ----
                kT = [kt_p.tile([P, T], F32R, tag="kt", name=f"kT{i}") for i in range(CT)]
                for of in range(CT):
                    for hp, src in ((0, aT), (1, aTp)):
                        pk = ps.tile([P, R], F32, tag="ps")
                        for i in range(CT):
                            nc.tensor.matmul(
                                pk[:],
                                wk[i][:, of * P : (of + 1) * P],
                                src[i][:],
                                start=(i == 0),
                                stop=(i == CT - 1),
                            )
                        dst = kT[of][:, hp * R : (hp + 1) * R]
                        if zq:
                            nc.scalar.activation(dst, pk[:], AF.Copy)
                        else:
                            nc.scalar.activation(
                                dst, pk[:], AF.Identity, bias=bk_sb[:, of : of + 1]
                            )

                # ---- V row-major -> V_aug [keys, 12, 65] with ones col ----
                v_aug = [
                    vaug_p.tile([P, NH, HD + 1], F32R, tag="vaug", name=f"va{k}")
                    for k in range(KT)
                ]
                for kt in range(KT):
                    src = aT if kt < RT else aTp
                    ksl = slice((kt % RT) * P, (kt % RT + 1) * P)
                    for nh0, nhn in ((0, 8), (8, 4)):
                        n0, nw = nh0 * HD, nhn * HD
                        pv = ps.tile([P, R], F32, tag="ps")
                        if not zq:
                            nc.tensor.matmul(
                                pv[:, :nw],
                                ones_row[:],
                                bv_row[:, n0 : n0 + nw],
                                start=True,
                                stop=False,
                            )
                        for i in range(CT):
                            nc.tensor.matmul(
                                pv[:, :nw],
                                src[i][:, ksl],
                                wv[i][:, n0 : n0 + nw],
                                start=(zq and i == 0),
                                stop=(i == CT - 1),
                            )
                        nc.vector.tensor_copy(
                            v_aug[kt][:, nh0 : nh0 + nhn, 0:HD],
                            pv[:, :nw].rearrange("p (h d) -> p h d", d=HD),
                        )
                    nc.vector.tensor_copy(
                        v_aug[kt][:, :, HD : HD + 1].rearrange("p h o -> p (h o)"),
                        ones_pp[:],
                    )

                # ---- attention per head ----
                yT = [b2k.tile([P, R], F32R, tag="b2k", bufs=24, name=f"yT{i}") for i in range(CT)]
                for hh in range(NH):
                    fo = hh // 2
                    psl = slice((hh % 2) * HD, (hh % 2) * HD + HD)
                    expT = []
                    for kt in range(KT):
                        et = attn_p.tile([P, R], F32R, tag="expT", bufs=8)
                        pscr = ps.tile([P, R], F32, tag="ps")
                        if kt % 2 == 0:
                            nc.tensor.matmul(
                                pscr[:],
                                kT[fo][psl, kt * P : (kt + 1) * P],
                                qT[fo][psl, :],
                                start=True,
                                stop=True,
                            )
                            nc.scalar.activation(et[:], pscr[:], AF.Exp)
                            for qs in (0, 2):
                                nc.vector.tensor_tensor(
                                    out=et[:, qs * P : (qs + 1) * P],
                                    in0=et[:, qs * P : (qs + 1) * P],
                                    in1=triu[:],
                                    op=ALU.mult,
                                )
                        else:
                            nc.vector.tensor_copy(et[:, 0:P], zero_t[:])
                            nc.vector.tensor_copy(et[:, 2 * P : 3 * P], zero_t[:])
                            nc.tensor.matmul(
                                pscr[:, P:R],
                                kT[fo][psl, kt * P : (kt + 1) * P],
                                qT[fo][psl, P:R],
                                start=True,
                                stop=True,
                            )
                            for qs in (1, 3):
                                sl_ = slice(qs * P, (qs + 1) * P)
                                nc.scalar.activation(et[:, sl_], pscr[:, sl_], AF.Exp)
                                nc.vector.tensor_tensor(
                                    out=et[:, sl_],
                                    in0=et[:, sl_],
                                    in1=triu[:],
                                    op=ALU.mult,
                                )
                        expT.append(et)
                    # AV: yT_h [65, R] = sum_kc (V_aug[kc] head slice as lhsT) @ expT
                    py = ps.tile([P, R], F32, tag="ps")
                    for j in range(KT):
                        nc.tensor.matmul(
                            py[: HD + 1, :],
                            v_aug[j][:, hh, :],
                            expT[j][:],
                            start=(j == 0),
                            stop=(j == KT - 1),
                        )
                    # divide by sums (row HD of py) via PE broadcast of recip
                    srow = small.tile([1, R], F32, tag="srow")
                    nc.scalar.activation(srow[:], py[HD : HD + 1, :], AF.Copy)
                    rrow = small.tile([1, R], F32R, tag="rrow")
                    with nc.allow_low_precision(reason="f32r recip feeds f32r matmul"):
                        nc.vector.reciprocal(rrow[:], srow[:])
                    pb = ps.tile([P, R], F32, tag="ps")
                    nc.tensor.matmul(pb[:], ones_row[:], rrow[:], start=True, stop=True)
                    sb_b = attn_p.tile([P, R], F32, tag="sb_b", bufs=2)
                    nc.scalar.activation(sb_b[:], pb[:], AF.Copy)
                    nc.vector.tensor_tensor(
                        out=yT[fo][psl, :],
                        in0=py[:HD, :],
                        in1=sb_b[:HD, :],
                        op=ALU.mult,
                    )

                # ---- proj + residual ----
                wp = [wqkv_p.tile([P, C], F32R, tag="wqkv", name=f"wp{i}") for i in range(CT)]
                for i in range(CT):
                    nc.sync.dma_start(wp[i][:], wp_in[layer, i * P : (i + 1) * P, :])
                if not zp:
                    bp_row = small.tile([1, C], F32R, tag="bp_row")
                    nc.sync.dma_start(bp_row[:], bp_in[lsl, :])
                for rt in range(RT):
                    for n0, n1 in ((0, 512), (512, 768)):
                        nw = n1 - n0
                        pp = ps.tile([P, R], F32, tag="ps")
                        if not zp:
                            nc.tensor.matmul(
                                pp[:, :nw],
                                ones_row[:],
                                bp_row[:, n0:n1],
                                start=True,
                                stop=False,
                            )
                        for i in range(CT):
                            nc.tensor.matmul(
                                pp[:, :nw],
                                yT[i][:, rt * P : (rt + 1) * P],
                                wp[i][:, n0:n1],
                                start=(zp and i == 0),
                                stop=(i == CT - 1),
                            )
                        nc.vector.tensor_tensor(
                            out=h[rt][:, n0:n1],
                            in0=h[rt][:, n0:n1],
                            in1=pp[:, :nw],
                            op=ALU.add,
                        )

                # ---- MLP ----
                mT = transpose_to_feat(rot, "mT", 6)

                b1_sb = small.tile([P, HT], F32, tag="b1_sb")
                nc.sync.dma_start(
                    b1_sb[:], b1_in[lsl, :].rearrange("o (f p) -> p (o f)", p=P)
                )
                h1T = [
                    b2k.tile([P, R], F32R, tag="b2k", bufs=24, name=f"h1T{i}")
                    for i in range(HT)
                ]
                for ofg in range(6):
                    w1c = [
                        w1_p.tile([P, 512], F32R, tag="w1c", name=f"w1c{i}")
                        for i in range(CT)
                    ]
                    for i in range(CT):
                        nc.sync.dma_start(
                            w1c[i][:],
                            w1_in[
                                layer, i * P : (i + 1) * P, ofg * 512 : (ofg + 1) * 512
                            ],
                        )
                    for oi in range(4):
                        of = ofg * 4 + oi
                        pf = ps.tile([P, R], F32, tag="ps")
                        for i in range(CT):
                            nc.tensor.matmul(
                                pf[:],
                                w1c[i][:, oi * P : (oi + 1) * P],
                                mT[i][:],
                                start=(i == 0),
                                stop=(i == CT - 1),
                            )
                        nc.scalar.activation(
                            h1T[of][:], pf[:], AF.Gelu, bias=b1_sb[:, of : of + 1]
                        )

                # ---- fc2: of-outer, two column passes, h += mlp ----
                if not z2:
                    b2_row = small.tile([1, C], F32R, tag="b2_row")
                    nc.sync.dma_start(b2_row[:], b2_in[lsl, :])
                for n0, n1 in ((0, 512), (512, 768)):
                    nw = n1 - n0
                    pacc = [psacc.tile([P, nw], F32, tag="psacc", name=f"pacc{_r}") for _r in range(RT)]
                    if not z2:
                        for rt in range(RT):
                            nc.tensor.matmul(
                                pacc[rt][:],
                                ones_row[:],
                                b2_row[:, n0:n1],
                                start=True,
                                stop=False,
                            )
                    w2 = [
                        w2_p.tile([P, 512], F32R, tag="w2", name=f"w2_{i}")
                        for i in range(HT)
                    ]
                    for i in range(HT):
                        nc.sync.dma_start(
                            w2[i][:, :nw], w2_in[layer, i * P : (i + 1) * P, n0:n1]
                        )
                        for rt in range(RT):
                            nc.tensor.matmul(
                                pacc[rt][:],
                                h1T[i][:, rt * P : (rt + 1) * P],
                                w2[i][:, :nw],
                                start=(z2 and i == 0),
                                stop=(i == HT - 1),
                            )
                    for rt in range(RT):
                        nc.vector.tensor_tensor(
                            out=h[rt][:, n0:n1],
                            in0=h[rt][:, n0:n1],
                            in1=pacc[rt][:],
                            op=ALU.add,
                        )

            for rt in range(RT):
                nc.sync.dma_start(out_d[rt * P : (rt + 1) * P, :], h[rt][:])

    nc.compile()
    return nc


# ------------------------ host side ------------------------

_CACHE = {}


def _prep_inputs(inputs):
    f32 = np.float32
    g1 = inputs["ln1_g"].astype(f32)[:, :, None]
    b1g = inputs["ln1_b"].astype(f32)
    g2 = inputs["ln2_g"].astype(f32)[:, :, None]
    b2g = inputs["ln2_b"].astype(f32)

    def fold(Wname, bname, g, b, scale=1.0):
        W = inputs[Wname].astype(f32)
        bias = inputs[bname].astype(f32)
        Weff = (g * W) * scale
        beff = (bias + np.einsum("lc,lcd->ld", b, W)) * scale
        return Weff.astype(f32), beff.astype(f32)

    wq, bq = fold("Wq", "bq", g1, b1g, 0.125)
    wk, bk = fold("Wk", "bk", g1, b1g)
    wv, bv = fold("Wv", "bv", g1, b1g)
    w1, b1 = fold("W1", "b1", g2, b2g)
    bp = inputs["bp"].astype(f32)
    b2 = inputs["b2"].astype(f32)

    # NOTE: f32r-declared params are rounded (~13-bit mantissa) by the
    # hardware path itself; passing raw fp32 avoids double rounding.
    common = {
        "wq": wq,
        "wk": wk,
        "wv": wv,
        "wp": inputs["Wp"].astype(f32),
        "w1": w1,
        "w2": inputs["W2"].astype(f32),
        "bq": bq,
        "bk": bk,
        "bv": bv,
        "bp": bp,
        "b1": b1,
        "b2": b2,
        "triu": np.triu(np.ones((P, P), np.float32)),
        "ident": np.eye(P, dtype=np.float32),
        "ones_row": np.ones((1, P), np.float32),
    }
    zq = bool(np.all(bq == 0) and np.all(bk == 0) and np.all(bv == 0))
    zp = bool(np.all(bp == 0))
    z2 = bool(np.all(b2 == 0))
    x = inputs["x"].astype(f32)
    shards = [
        np.ascontiguousarray(x[c // 2, (c % 2) * R : (c % 2 + 1) * R, :])
        for c in range(NCORES)
    ]
    return common, shards, (zq, zp, z2)


def get_nc(flags):
    if flags not in _CACHE:
        _CACHE[flags] = build(*flags)
    return _CACHE[flags]


def kernel(**inputs):
    from concourse.bass_utils import run_bass_kernel_spmd

    common, shards, flags = _prep_inputs(inputs)
    nc = get_nc(flags)
    in_maps = [dict(common, x=shards[c]) for c in range(NCORES)]
    res = run_bass_kernel_spmd(nc, in_maps, list(range(NCORES)), trace=False)
    out = np.empty((B, T, C), np.float32)
    for c in range(NCORES):
        out[c // 2, (c % 2) * R : (c % 2 + 1) * R, :] = res.results[c]["out"]
    return out


if __name__ == "__main__":
    nc = build(True, True, True)
    print("build+compile OK; instructions:", len(nc.m.functions[0].blocks[0].instructions) if nc.m.functions else "?")

